# revision 15
# baseline (speedup 1.0000x reference)
"""Causal self-attention (B=4, T=2048, C=2048, H=16, RoPE) on 8 trn2 cores.

Sharding: core c -> (batch b = c//2, head-group g = c%2), 8 heads per core.
Each core computes y_partial[b] = attn_heads(g) @ W_proj[rows(g)]; the host
sums the two partials per batch.

Fused single-pass design (v2): x stays resident in SBUF (bf16); per head,
the QKV projection + RoPE produce qT/kT/v on-chip and attention runs
immediately — no DRAM round trip for q/k/v or O. The whole datapath is bf16
(measured end-to-end rel err ~5e-3 vs the 2e-2 budget) with fp32 PSUM
accumulation and fp32 softmax row-sums split across DVE+Pool chains.
Normalization uses a PE broadcast matmul (ones[1,128] x inv[1,512]) instead
of a DRAM broadcast DMA, and is emitted one q-block late so the PE never
waits on the DVE sum/reciprocal chain. The output projection reads O^T
directly from SBUF, reusing the attention's PSUM pool.
"""
import sys

sys.path.insert(0, "/opt/trn_rl_repo")

import numpy as np

B, T, C, H, D = 4, 2048, 2048, 16, 128
G = 2                      # head groups (tensor-parallel dim)
HG = H // G                # heads per core = 8
CG = HG * D                # channels per group = 1024
P = 128
NQ = T // 512              # q chunks of 512
KO = C // P                # contraction chunks = 16
ROPE_BASE = 10000.0
SCALE = 1.0 / float(np.sqrt(D))
N_CORES = 8

_cached = None


def _build_program(reps=1, phases="all", variant="full", bench_mode=False):
    import concourse.bass as bass
    import concourse.tile as tile
    from concourse import bacc, mybir

    f32 = mybir.dt.float32
    f32r = mybir.dt.float32r
    bf16 = mybir.dt.bfloat16
    Exp = mybir.ActivationFunctionType.Exp

    nc = bacc.Bacc()

    xq_d = nc.declare_dram_parameter("xq", [4, P, KO, 512], bf16, isOutput=False)
    wq_d = nc.declare_dram_parameter("wq", [HG, P, KO, D], bf16, isOutput=False)
    wk_d = nc.declare_dram_parameter("wk", [HG, P, KO, D], bf16, isOutput=False)
    wv_d = nc.declare_dram_parameter("wv", [HG, P, KO, D], bf16, isOutput=False)
    wp_d = nc.declare_dram_parameter("wp", [4, P, HG, 512], bf16, isOutput=False)
    cos_d = nc.declare_dram_parameter("cosT", [P, T], bf16, isOutput=False)
    sin_d = nc.declare_dram_parameter("sinT", [P, T], bf16, isOutput=False)
    swp_d = nc.declare_dram_parameter("swapT", [P, P], f32, isOutput=False)
    ones_d = nc.declare_dram_parameter("ones", [P, 1], f32, isOutput=False)
    onesb_d = nc.declare_dram_parameter("onesb", [1, P], f32, isOutput=False)
    mask_d = nc.declare_dram_parameter("masks", [P, 4, 512], bf16, isOutput=False)
    if bench_mode:
        y_d = nc.dram_tensor("y_scratch", [T, C], f32)
        tok_d = nc.declare_dram_parameter("tok", [P, P], f32, isOutput=True)
    else:
        y_d = nc.declare_dram_parameter("y", [T, C], f32, isOutput=True)
        tok_d = None

    for _rep in range(reps):
        with tile.TileContext(nc) as tc:
            with tc.tile_pool(name="const", bufs=1) as cp, \
                 tc.tile_pool(name="xres", bufs=1) as xp, \
                 tc.tile_pool(name="wqk", bufs=2) as wqkp, \
                 tc.tile_pool(name="qk", bufs=2) as qkp, \
                 tc.tile_pool(name="vres", bufs=2) as vrp, \
                 tc.tile_pool(name="rope", bufs=2) as rp, \
                 tc.tile_pool(name="att", bufs=4) as atp, \
                 tc.tile_pool(name="sums", bufs=1) as smp, \
                 tc.tile_pool(name="ou", bufs=2) as oup, \
                 tc.tile_pool(name="oall", bufs=1) as oap, \
                 tc.tile_pool(name="wpp", bufs=2) as wpp, \
                 tc.tile_pool(name="yp", bufs=2) as yp, \
                 tc.tile_pool(name="ps", bufs=1, space="PSUM") as ps:
                cosT = cp.tile([P, T], bf16)
                sinT = cp.tile([P, T], bf16)
                swpT = cp.tile([P, P], f32r)
                ones = cp.tile([P, 1], f32r)
                onesb = cp.tile([1, P], f32r)
                masks = cp.tile([P, 4, 512], bf16)
                o_all = oap.tile([P, HG, T], bf16)
                xt = [xp.tile([P, KO, 512], bf16, tag=f"x{i}", name=f"x{i}")
                      for i in range(4)]

                def load_x(i, ko_lo, ko_hi):
                    ks = slice(ko_lo, ko_hi)
                    nc.sync.dma_start(xt[i][:, ks, :], xq_d.ap()[i][:, ks, :])

                def load_w(h, w_d, tag):
                    wt = wqkp.tile([P, KO, D], bf16, tag=tag, name=tag)
                    nc.sync.dma_start(wt[:], w_d.ap()[h])
                    return wt

                # ---- DMA prelude: critical-path-first ----
                w_tiles = {}
                w_tiles[(0, "wq")] = load_w(0, wq_d, "wq")
                load_x(0, 0, 4)
                nc.sync.dma_start(swpT[:], swp_d.ap().bitcast(f32r))
                load_x(0, 4, 16)
                nc.sync.dma_start(cosT[:], cos_d.ap())
                nc.sync.dma_start(sinT[:], sin_d.ap())
                w_tiles[(0, "wk")] = load_w(0, wk_d, "wk")
                w_tiles[(0, "wv")] = load_w(0, wv_d, "wv")
                load_x(1, 0, 16)
                nc.sync.dma_start(masks[:], mask_d.ap())
                nc.sync.dma_start(ones[:], ones_d.ap().bitcast(f32r))
                nc.sync.dma_start(onesb[:], onesb_d.ap().bitcast(f32r))
                load_x(2, 0, 16)
                load_x(3, 0, 16)

                # deferred normalization: (ptsum_r, ou_tile, h, qb) emitted
                # later so PE never waits on the DVE sum/recip chain
                pend = []

                def emit_norm_psn(item):
                    ptsum_r, ou_t, h_, qb_ = item
                    ps_n = ps.tile([1, 512], f32, tag="n", bufs=1, name="psn")
                    nc.tensor.matmul(ps_n[:], ones[:], ptsum_r[:],
                                     start=True, stop=True)
                    inv = smp.tile([1, 512], f32r, tag="inv", bufs=1, name="inv")
                    with nc.allow_low_precision(reason="1/n in f32r for bcast matmul"):
                        nc.vector.reciprocal(inv[:], ps_n[:])
                    return inv

                def emit_norm_psb(item, inv):
                    ptsum_r, ou_t, h_, qb_ = item
                    ps_b = ps.tile([P, 512], f32, tag="b", bufs=1, name="psb")
                    nc.tensor.matmul(ps_b[:], onesb[:], inv[:],
                                     start=True, stop=True)
                    nc.vector.tensor_mul(
                        o_all[:, h_, qb_ * 512:(qb_ + 1) * 512], ou_t[:], ps_b[:])

                def flush_pend_between(chunks):
                    """Interleave pending norm emissions between PE chunks:
                    psn for item i lands after chunk i, its psb after chunk
                    i+1, so the PE has dense work while DVE catches up."""
                    items = pend[:]
                    del pend[:]
                    invs = {}
                    done = set()
                    for i, ch in enumerate(chunks):
                        ch()
                        if i < len(items):
                            invs[i] = emit_norm_psn(items[i])
                        if i - 1 in invs and i - 1 not in done:
                            emit_norm_psb(items[i - 1], invs[i - 1])
                            done.add(i - 1)
                    for j in range(len(items)):
                        if j not in invs:
                            invs[j] = emit_norm_psn(items[j])
                        if j not in done:
                            emit_norm_psb(items[j], invs[j])
                            done.add(j)

                def mk_qk(ql, wt, dst):
                    def emit():
                        psq = ps.tile([P, 512], f32, tag="sp", bufs=4,
                                      name="psq")
                        for ki in range(KO):
                            nc.tensor.matmul(psq[:], wt[:, ki, :],
                                             xt[ql][:, ki, :],
                                             start=(ki == 0),
                                             stop=(ki == KO - 1))
                        raw = rp.tile([P, 512], f32r, tag="raw", name="raw")
                        nc.scalar.copy(raw[:], psq[:])
                        ps2 = ps.tile([P, 512], f32, tag="sw", bufs=1,
                                      name="ps2")
                        nc.tensor.matmul(ps2[:], swpT[:], raw[:],
                                         start=True, stop=True)
                        tgl = slice(ql * 512, (ql + 1) * 512)
                        tA = rp.tile([P, 512], f32, tag="tA", name="tA")
                        nc.vector.tensor_mul(tA[:], raw[:], cosT[:, tgl])
                        tB = rp.tile([P, 512], f32, tag="tB", name="tB")
                        nc.vector.tensor_mul(tB[:], ps2[:], sinT[:, tgl])
                        nc.vector.tensor_add(dst[:, tgl], tA[:], tB[:])
                    return emit

                def mk_v(ql, wvt, vh):
                    def emit():
                        for tb in range(4):
                            psv = ps.tile([P, D], f32, tag="sp", bufs=4,
                                          name="psv")
                            for ki in range(KO):
                                nc.tensor.matmul(
                                    psv[:],
                                    xt[ql][:, ki, tb * P:(tb + 1) * P],
                                    wvt[:, ki, :],
                                    start=(ki == 0), stop=(ki == KO - 1))
                            nc.vector.tensor_copy(vh[:, ql * 4 + tb, :],
                                                  psv[:])
                    return emit

                def emit_attn_qb(h_, qt_, kt_, vh_, qb):
                    nkb = 4 * (qb + 1)
                    ps_o = ps.tile([P, 512], f32, tag="o", bufs=1, name="pso")
                    sA = smp.tile([P, 512], f32, tag="sA", name="sA")
                    sB = smp.tile([P, 512], f32, tag="sB", name="sB")
                    inv_prev = None
                    prev_item = pend[-1] if pend else None
                    for kb in range(nkb):
                        ps_s = ps.tile([P, 512], f32, tag="sp", bufs=4,
                                       name="pss")
                        nc.tensor.matmul(ps_s[:], kt_[:, kb * P:(kb + 1) * P],
                                         qt_[:, qb * 512:(qb + 1) * 512],
                                         start=True, stop=True)
                        if prev_item is not None:
                            if kb == 1:
                                inv_prev = emit_norm_psn(prev_item)
                            elif kb == 3:
                                emit_norm_psb(prev_item, inv_prev)
                                pend.remove(prev_item)
                                prev_item = None
                        pt = atp.tile([P, 512], bf16, tag="pt", name="pt")
                        nc.scalar.activation(pt[:], ps_s[:], Exp, scale=SCALE)
                        j = kb - 4 * qb
                        if j >= 0:
                            ptm = atp.tile([P, 512], bf16, tag="ptm",
                                           bufs=2, name="ptm")
                            nc.vector.tensor_mul(ptm[:], pt[:], masks[:, j, :])
                            pt = ptm
                        nc.tensor.matmul(ps_o[:], vh_[:, kb, :], pt[:],
                                         start=(kb == 0),
                                         stop=(kb == nkb - 1))
                        tgt, eng = (sA, nc.vector) if kb % 2 == 0 else \
                                   (sB, nc.gpsimd)
                        if kb < 2:
                            eng.tensor_copy(tgt[:], pt[:])
                        else:
                            eng.tensor_add(tgt[:], tgt[:], pt[:])
                    ptsum_r = smp.tile([P, 512], f32r, tag="sr", bufs=2,
                                       name="ptsum_r")
                    nc.vector.tensor_add(ptsum_r[:], sA[:], sB[:])
                    ou_t = oup.tile([P, 512], bf16, tag="ou", name="ou")
                    nc.scalar.copy(ou_t[:], ps_o[:])
                    pend.append((ptsum_r, ou_t, h_, qb))

                # ---- head pipeline: attention of head h-1 is interleaved
                # into the projection emission of head h so the in-order PE
                # always has dense matmul work while ACT chews the exps ----
                prev_attn = None
                for h in range(HG):
                    if h + 1 < HG:
                        for (w_d, tag) in ((wq_d, "wq"), (wk_d, "wk"),
                                           (wv_d, "wv")):
                            w_tiles[(h + 1, tag)] = load_w(h + 1, w_d, tag)
                    wqt = w_tiles.pop((h, "wq"))
                    wkt = w_tiles.pop((h, "wk"))
                    wvt = w_tiles.pop((h, "wv"))
                    qt = qkp.tile([P, T], bf16, tag="qt", name="qt")
                    kt = qkp.tile([P, T], bf16, tag="kt", name="kt")
                    vh = vrp.tile([P, KO, D], bf16, tag="vh", name="vh")

                    chunks = []
                    for ql in range(4):
                        chunks.append(mk_qk(ql, wqt, qt))
                        chunks.append(mk_qk(ql, wkt, kt))
                        chunks.append(mk_v(ql, wvt, vh))

                    if prev_attn is None:
                        for c in chunks:
                            c()
                    else:
                        ph, pqt, pkt, pvh = prev_attn
                        emit_attn_qb(ph, pqt, pkt, pvh, 0)
                        chunks[0]()
                        emit_attn_qb(ph, pqt, pkt, pvh, 1)
                        chunks[1]()
                        chunks[2]()
                        emit_attn_qb(ph, pqt, pkt, pvh, 2)
                        chunks[3]()
                        chunks[4]()
                        chunks[5]()
                        emit_attn_qb(ph, pqt, pkt, pvh, 3)
                        for c in chunks[6:]:
                            c()
                    prev_attn = (h, qt, kt, vh)

                # ---- tail: attention of head 7 interleaved with the first
                # output-projection column; remaining columns run dense ----
                def mk_y_part(co_, wpc_, qc_lo, qc_hi):
                    for qc in range(qc_lo, qc_hi):
                        psy = ps.tile([P, 512], f32, tag="sp", bufs=4,
                                      name="psy")
                        for hh in range(HG):
                            nc.tensor.matmul(
                                psy[:],
                                o_all[:, hh, qc * P:(qc + 1) * P],
                                wpc_[:, hh, :],
                                start=(hh == 0), stop=(hh == HG - 1))
                        ysb = yp.tile([P, 512], f32, tag="ysb", name="ysb")
                        nc.scalar.copy(ysb[:], psy[:])
                        nc.sync.dma_start(
                            y_d.ap()[qc * P:(qc + 1) * P,
                                     co_ * 512:(co_ + 1) * 512], ysb[:])
                        if (bench_mode and co_ == C // 512 - 1
                                and qc == T // P - 1):
                            nc.sync.dma_start(tok_d.ap(), ysb[:, :P])

                ph, pqt, pkt, pvh = prev_attn
                wp_tiles = []
                for co in range(2):
                    wpc = wpp.tile([P, HG, 512], bf16, tag="wp", name="wpc")
                    nc.sync.dma_start(wpc[:], wp_d.ap()[co])
                    wp_tiles.append(wpc)
                emit_attn_qb(ph, pqt, pkt, pvh, 0)
                emit_attn_qb(ph, pqt, pkt, pvh, 1)   # norm(7,qb0) inline
                mk_y_part(0, wp_tiles[0], 0, 4)
                emit_attn_qb(ph, pqt, pkt, pvh, 2)   # norm(7,qb1)
                mk_y_part(0, wp_tiles[0], 4, 8)
                emit_attn_qb(ph, pqt, pkt, pvh, 3)   # norm(7,qb2)
                mk_y_part(0, wp_tiles[0], 8, 12)
                flush_pend_between([])               # norm(7,qb3)
                mk_y_part(0, wp_tiles[0], 12, 16)
                for co in range(1, C // 512):
                    if co >= 2:
                        wpc = wpp.tile([P, HG, 512], bf16, tag="wp",
                                       name="wpc")
                        nc.sync.dma_start(wpc[:], wp_d.ap()[co])
                    else:
                        wpc = wp_tiles[co]
                    mk_y_part(co, wpc, 0, 16)

    nc.finalize()
    return nc


def _host_tables():
    import ml_dtypes
    thetas = 1.0 / (ROPE_BASE ** (np.arange(0, D, 2, dtype=np.float32) / D))
    t = np.arange(T, dtype=np.float32)
    freqs = t[None, :] * thetas[:, None]                     # [64, T]
    cosT = np.repeat(np.cos(freqs), 2, axis=0).astype(ml_dtypes.bfloat16)
    sinT = np.repeat(np.sin(freqs), 2, axis=0).astype(ml_dtypes.bfloat16)
    swapT = np.zeros((P, P), np.float32)
    for i in range(0, P, 2):
        swapT[i, i + 1] = 1.0
        swapT[i + 1, i] = -1.0
    ones = np.ones((P, 1), np.float32)
    onesb = np.ones((1, P), np.float32)
    ki = np.arange(P)[:, None]
    qi = np.arange(512)[None, :]
    masks = np.stack([(ki + 128 * j <= qi) for j in range(4)],
                     axis=1).astype(ml_dtypes.bfloat16)  # [128, 4, 512]
    return cosT, sinT, swapT, ones, onesb, np.ascontiguousarray(masks)


class _Runner:
    """Compile the bass program to a PJRT executable once; rerun cheaply."""

    def __init__(self, nc):
        import jax
        from jax.sharding import Mesh, PartitionSpec
        try:
            from jax.experimental.shard_map import shard_map
        except ImportError:
            from jax import shard_map
        from concourse import bass2jax, mybir

        bass2jax.install_neuronx_cc_hook()
        self.jax = jax
        self.nc = nc
        assert nc.dbg_addr is None or not nc.dbg_callbacks
        partition_name = (nc.partition_id_tensor.name
                          if nc.partition_id_tensor else None)

        in_names, out_names, out_avals, zero_shapes = [], [], [], []
        for alloc in nc.m.functions[0].allocations:
            if not isinstance(alloc, mybir.MemoryLocationSet):
                continue
            name = alloc.memorylocations[0].name
            if alloc.kind == "ExternalInput":
                if name != partition_name and name != (
                        nc.dbg_addr.name if nc.dbg_addr else None):
                    in_names.append(name)
            elif alloc.kind == "ExternalOutput":
                shape = tuple(alloc.tensor_shape)
                dtype = mybir.dt.np(alloc.dtype)
                out_names.append(name)
                out_avals.append(jax.core.ShapedArray(shape, dtype))
                zero_shapes.append((shape, dtype))
        self.in_names, self.out_names = in_names, out_names
        self.out_avals, self.zero_shapes = out_avals, zero_shapes
        n_params, n_outs = len(in_names), len(out_names)
        self.n_params = n_params

        all_names = list(in_names) + list(out_names)
        if nc.dbg_addr is not None:
            all_names.append(nc.dbg_addr.name)
        if partition_name is not None:
            all_names.append(partition_name)

        def _body(*args):
            operands = list(args)
            if nc.dbg_addr is not None:
                operands.append(jax.numpy.zeros((1, 2), "uint32"))
            if partition_name is not None:
                operands.append(bass2jax.partition_id_tensor())
            outs = bass2jax._bass_exec_p.bind(
                *operands,
                out_avals=tuple(out_avals),
                in_names=tuple(all_names),
                out_names=tuple(out_names),
                lowering_input_output_aliases=(),
                sim_require_finite=True,
                sim_require_nnan=True,
                nc=nc,
            )
            return tuple(outs)

        devices = jax.devices()[:N_CORES]
        self.mesh = Mesh(np.asarray(devices), ("core",))
        self.pspec = PartitionSpec("core")
        in_specs = (self.pspec,) * (n_params + n_outs)
        out_specs = (self.pspec,) * n_outs
        donate = tuple(range(n_params, n_params + n_outs))
        self.fn = jax.jit(
            shard_map(_body, mesh=self.mesh, in_specs=in_specs,
                      out_specs=out_specs, check_rep=False),
            donate_argnums=donate, keep_unused=True)

    def concat_inputs(self, in_maps):
        return [np.concatenate([np.asarray(in_maps[c][n])
                                for c in range(N_CORES)], axis=0)
                for n in self.in_names]

    def device_inputs(self, concat_in):
        from jax.sharding import NamedSharding
        sh = NamedSharding(self.mesh, self.pspec)
        return [self.jax.device_put(a, sh) for a in concat_in]

    def zeros(self, on_device=False):
        zs = [np.zeros((N_CORES * s[0], *s[1:]), d) for s, d in self.zero_shapes]
        if on_device:
            from jax.sharding import NamedSharding
            sh = NamedSharding(self.mesh, self.pspec)
            zs = [self.jax.device_put(z, sh) for z in zs]
        return zs

    def run(self, args):
        out_arrs = self.fn(*args)
        return [
            {n: np.asarray(out_arrs[i]).reshape(N_CORES, *self.out_avals[i].shape)[c]
             for i, n in enumerate(self.out_names)}
            for c in range(N_CORES)
        ]


_runner = None


def _get_runner():
    global _cached, _runner
    if _runner is None:
        if _cached is None:
            _cached = _build_program()
        _runner = _Runner(_cached)
    return _runner


def _make_in_maps(x, W_qkv, W_proj):
    import ml_dtypes
    bf = ml_dtypes.bfloat16
    cosT, sinT, swapT, ones, onesb, masks = _host_tables()
    in_maps = []
    for c in range(N_CORES):
        b, g = c // G, c % G
        cols = slice(g * CG, (g + 1) * CG)
        xT = x[b].T  # [C, T]
        wq = W_qkv[:, 0 * C:1 * C][:, cols]
        wk = W_qkv[:, 1 * C:2 * C][:, cols]
        wv = W_qkv[:, 2 * C:3 * C][:, cols]
        wpm = W_proj[g * CG:(g + 1) * CG, :]
        in_maps.append({
            # [C, T] -> [tq, p, ko, 512]
            "xq": np.ascontiguousarray(
                xT.reshape(KO, P, 4, 512).transpose(2, 1, 0, 3)).astype(bf),
            # [C, CG] -> [h, p, ko, D]
            "wq": np.ascontiguousarray(
                wq.reshape(KO, P, HG, D).transpose(2, 1, 0, 3)).astype(bf),
            "wk": np.ascontiguousarray(
                wk.reshape(KO, P, HG, D).transpose(2, 1, 0, 3)).astype(bf),
            "wv": np.ascontiguousarray(
                wv.reshape(KO, P, HG, D).transpose(2, 1, 0, 3)).astype(bf),
            # [CG, C] -> [co, p, hb, 512]
            "wp": np.ascontiguousarray(
                wpm.reshape(HG, P, 4, 512).transpose(2, 1, 0, 3)).astype(bf),
            "cosT": cosT, "sinT": sinT, "swapT": swapT,
            "ones": ones, "onesb": onesb, "masks": masks,
        })
    return in_maps


def kernel(x, W_qkv, W_proj):
    x = np.asarray(x, dtype=np.float32)
    W_qkv = np.asarray(W_qkv, dtype=np.float32)
    W_proj = np.asarray(W_proj, dtype=np.float32)

    r = _get_runner()
    concat_in = r.concat_inputs(_make_in_maps(x, W_qkv, W_proj))
    results = r.run(concat_in + r.zeros())
    out = np.empty((B, T, C), np.float32)
    for b in range(B):
        out[b] = results[2 * b]["y"] + results[2 * b + 1]["y"]
    return out
